# revision 71
# baseline (speedup 1.0000x reference)
"""Trainium2 Bass kernel for the BalSCL/SSL balanced supervised-contrastive loss.

Distribution: data-parallel over the 8192 anchor rows, 1024 rows per core on
8 NeuronCores.  Each core returns two partial-loss scalars (the conf-weighted
sum of ln S_i over its two 512-row chunks); the host combines them with the
host-computed linear (mean-positive-logit) term and conf denominator.

Math (restructured from the reference, analytically identical):
  N = 8292 columns (8192 anchors + 100 class centers), all unit-norm.
  The row-max subtraction in the reference cancels analytically, so
    loss_i = ln(S_i) - (10/m_i) * Sm_i
  with
    S_i  = sum_{j != i} exp(10 * f_i . g_j) / (cc_j - [lab_j == lab_i])
    Sm_i = sum_{j != i, lab_j == lab_i} f_i . g_j      (host, exact f64)
    m_i  = cc[lab_i] - 1
  Device work per core: raw logits r = fTg.T @ fTc (bf16 PE), elementwise
  exp(10 r) quantized to fp8e5m2, and per-class sums E[c,i] via fp8 DoubleRow
  matmuls (two 128-row j-tiles per PE pass).  S_i = sum_c W2c[c,i] E[c,i] - dg_i
  where W2c folds the per-class balanced weights and the conf mask, and dg
  subtracts the diagonal (j == i) fp8 term bit-exactly.

  The exp work is split between the Scalar engine (true spline exp, RNE to
  fp8e5m2 -- hardware-validated exact) and the Vector engine (Schraudolph
  trick: y = round(r*40/ln2 + B) as int8, bit-reinterpreted as fp8e5m2; B
  calibrated so the mean log error over the logit distribution vanishes).
  j-tile pair p of chunk k goes to Scalar iff (p+k) is even, which maps each
  core's diagonal 256-row blocks to a fixed generator ([ACT, DVE, DVE, ACT]
  over the core's four blocks), so the diagonal correction -e1 can be fully
  precomputed on the host (replicating both generators bit-exactly in numpy)
  and shipped as a bf16 row that the srow matmul folds in via an extra
  contraction row; Ln then reads the PSUM result directly and accumulates
  the conf-masked partial loss.

  The E matmuls use DoubleRowSwInterleave (weights pre-interleaved column-
  reversed on the host) rather than plain DoubleRow: the contiguous weight
  read keeps the 256-column LDWEIGHTS fully hidden under the raw-matmul
  stream (plain DoubleRow exposed ~190 ns per E matmul).
"""

import os
import sys

sys.path.insert(0, "/opt/trn_rl_repo")

import numpy as np
import ml_dtypes

import concourse.bass as bass  # noqa: F401
import concourse.bacc as bacc
import concourse.tile as tile
from concourse import mybir
from concourse.bass_utils import run_bass_kernel_spmd

F32 = mybir.dt.float32
BF16 = mybir.dt.bfloat16
FP8 = mybir.dt.float8e5
I8 = mybir.dt.int8
BF = ml_dtypes.bfloat16
F8NP = ml_dtypes.float8_e5m2
AF = mybir.ActivationFunctionType
ALU = mybir.AluOpType
DR = mybir.MatmulPerfMode.DoubleRow

B2, C, D = 8192, 100, 128
TEMP = 0.1
N = B2 + C
TILES = 66                 # 65 real j-tiles + 1 zero pad (for pairing)
PAIRS = TILES // 2         # 33
NPAD2 = TILES * 128        # 8448
SWI = os.environ.get("KB_SWI", "1") == "1"
CP = 128 if SWI else 112   # padded class count per weight tile
CORES = 8
R = B2 // CORES            # 1024 rows per core
CH = 512                   # i-chunk width (one fp32 PSUM bank)
A_TRICK = 40.0 / np.log(2.0)   # 57.70780163555855
B_TRICK = 59.8                 # calibrated: zero mean log-error (see sim)

_NC_CACHE = {}

EP_BUFS = int(os.environ.get("KB_EPBUFS", "6"))
ELAG = int(os.environ.get("KB_ELAG", "3"))

if os.environ.get("KB_LDWOPT", "0") == "1":
    import concourse.bass_utils as _bu

    _orig_rc = _bu.run_command

    def _rc(cmd, *a, **k):
        cmd = [
            "--enable-ldw-opt=true" if c == "--enable-ldw-opt=false" else c
            for c in cmd
        ]
        return _orig_rc(cmd, *a, **k)

    _bu.run_command = _rc

# Combined exp+ln activation-table set: a single ACT_TABLE_LOAD.
_orig_gat = bacc.get_activation_tables


def _gat_combined(arch):
    tabs = _orig_gat(arch)
    out = {}
    for name, funcs in tabs.items():
        if name in ("exp_and_others", "exp_and_friends", "natural_log"):
            out[name] = set()  # keep position (set ids are positional)
        else:
            out[name] = funcs
    return out


def _build_nc():
    bacc.get_activation_tables = _gat_combined
    try:
        return _build_nc_inner()
    finally:
        bacc.get_activation_tables = _orig_gat


def _build_nc_inner():
    nc = bacc.Bacc()

    fTg = nc.dram_tensor("fTg", [D, NPAD2], BF16, kind="ExternalInput")
    TAg = nc.dram_tensor("TAg", [128, TILES * CP], FP8, kind="ExternalInput")
    fTc = nc.dram_tensor("fTc", [D, R], BF16, kind="ExternalInput")
    W2c = nc.dram_tensor("W2c", [C, R], BF16, kind="ExternalInput")
    ne1 = nc.dram_tensor("ne1", [1, R], BF16, kind="ExternalInput")
    outd = nc.dram_tensor("out", [1, R], F32, kind="ExternalOutput")

    with tile.TileContext(nc) as tc:
        with (
            tc.tile_pool(name="consts", bufs=1) as cp,
            tc.tile_pool(name="expp", bufs=EP_BUFS) as ep,
            tc.tile_pool(name="rawp", bufs=3, space="PSUM") as rp,
            tc.tile_pool(name="epsp", bufs=1, space="PSUM") as pp,
        ):
            # ---------------- input loads (sync queue: big streams) --------
            s_fTc = cp.tile([D, R], BF16)
            s_fTg = cp.tile([D, NPAD2], BF16)
            s_TAg = cp.tile([128, TILES * CP], FP8)
            nc.sync.dma_start(out=s_fTc[:, 0:CH], in_=fTc[:, 0:CH])
            for a, b in (
                (0, 384), (384, 1152), (1152, 2816),
                (2816, 5376), (5376, NPAD2),
            ):
                nc.sync.dma_start(out=s_fTg[:, a:b], in_=fTg[:, a:b])

            # gpsimd queue: memsets + small/medium loads
            s_scr = cp.tile([128, CH], BF16)
            nc.gpsimd.memset(s_scr, 1.0)
            s_ones_bf = cp.tile([128, 1], BF16)
            nc.gpsimd.memset(s_ones_bf, 1.0)
            PW = 2 * CP
            for pa, pb in ((0, 2), (2, 5), (5, 12)):
                nc.gpsimd.dma_start(
                    out=s_TAg[:, pa * PW : pb * PW], in_=TAg[:, pa * PW : pb * PW]
                )
            nc.gpsimd.dma_start(out=s_fTc[:, CH:R], in_=fTc[:, CH:R])
            for pa, pb in ((12, 20), (20, PAIRS)):
                nc.gpsimd.dma_start(
                    out=s_TAg[:, pa * PW : pb * PW], in_=TAg[:, pa * PW : pb * PW]
                )
            s_W2c = cp.tile([C, R], BF16)
            nc.gpsimd.dma_start(out=s_W2c, in_=W2c[:])

            # ---------------- PE warm-up (HAM un-throttle) -----------------
            for w in range(3):
                warmPS = rp.tile([128, 2 * CH], F32, name=f"warm{w}", tag="raw")
                for h in (0, 1):
                    nc.tensor.matmul(
                        warmPS[:, h * CH : (h + 1) * CH],
                        lhsT=s_scr[:, 0:128], rhs=s_scr, start=True, stop=True,
                    )


            # W2E tiles carry an extra row: row C holds -e1 in bf16 (host
            # computed, DMA'd in), so the srow matmul computes
            # S = sum_c W2c*E - e1 directly and Ln reads the PSUM result.
            w2e_t = [
                cp.tile([C + 1, CH], BF16, name=f"W2E{k}", tag=f"W2E{k}")
                for k in (0, 1)
            ]
            for k in (0, 1):
                nc.gpsimd.dma_start(
                    out=w2e_t[k][C : C + 1, :], in_=ne1[:, k * CH : (k + 1) * CH]
                )

            # ---------------- main pipeline --------------------------------
            exps_t = {}

            def raw_pair(k, p):
                rawPS = rp.tile([128, 2 * CH], F32, name=f"raw{k}_{p}", tag="raw")
                for q in (0, 1):
                    t = 2 * p + q
                    nc.tensor.matmul(
                        rawPS[:, q * CH : (q + 1) * CH],
                        lhsT=s_fTg[:, 128 * t : 128 * (t + 1)],
                        rhs=s_fTc[:, k * CH : (k + 1) * CH],
                        start=True,
                        stop=True,
                    )
                return rawPS

            def exp_pair(k, p, rawPS):
                exps = ep.tile([128, 2 * CH], FP8, name=f"exps{k}_{p}", tag="exps")
                if (p + k) % 2 == 0:
                    nc.scalar.activation(
                        out=exps, in_=rawPS, func=AF.Exp, scale=1.0 / TEMP
                    )
                else:
                    nc.vector.tensor_scalar(
                        out=exps[:].bitcast(I8), in0=rawPS,
                        scalar1=A_TRICK, scalar2=B_TRICK,
                        op0=ALU.mult, op1=ALU.add,
                    )
                exps_t[(k, p)] = exps

            EPS_t = [None, None]

            def e_mm(k, p):
                if EPS_t[k] is None:
                    EPS_t[k] = pp.tile([CP, CH], F32, name=f"EPS{k}", tag="EPS")
                exps = exps_t.pop((k, p))
                w_ap = s_TAg[:, 2 * CP * p : 2 * CP * (p + 1)]
                if not SWI:
                    w_ap = w_ap.rearrange("a (two c) -> a two c", two=2)
                nc.tensor.matmul(
                    EPS_t[k],
                    lhsT=w_ap,
                    rhs=exps[:].rearrange("a (two n) -> a two n", two=2),
                    start=(p == 0),
                    stop=(p == PAIRS - 1),
                    perf_mode=(
                        mybir.MatmulPerfMode.DoubleRowSwInterleave if SWI else DR
                    ),
                )

            def mk_w2e(k):
                nc.vector.tensor_mul(
                    w2e_t[k][0:C, :], EPS_t[k][0:C, :],
                    s_W2c[:, k * CH : (k + 1) * CH],
                )

            srow_t = [None, None]

            def mk_srow(k):
                srowPS = pp.tile([1, CH], F32, name=f"srowPS{k}", tag="srow")
                nc.tensor.matmul(
                    srowPS, lhsT=s_ones_bf[0 : C + 1, :], rhs=w2e_t[k],
                    start=True, stop=True,
                )
                srow_t[k] = srowPS

            def mk_out(k):
                # ship per-row S to the host; ln happens there in f64
                s_S = cp.tile([1, CH], F32, name=f"sS{k}", tag=f"sS{k}")
                nc.vector.tensor_copy(s_S, srow_t[k])
                nc.sync.dma_start(
                    out=outd[:, k * CH : (k + 1) * CH], in_=s_S
                )

            sca_after = {(1, 9): [lambda: mk_out(0)]}

            for k in (0, 1):
                for p in range(PAIRS):
                    rawPS = raw_pair(k, p)
                    if k == 1 and p == 3:
                        mk_srow(0)
                    exp_pair(k, p, rawPS)
                    for fn in sca_after.pop((k, p), ()):
                        fn()
                    if p >= ELAG:
                        e_mm(k, p - ELAG)
                for p in range(PAIRS - ELAG, PAIRS):
                    e_mm(k, p)
                mk_w2e(k)

            # ---------------- tail ----------------
            mk_srow(1)
            mk_out(1)

    nc.finalize()
    return nc


def _get_nc():
    if "nc" not in _NC_CACHE:
        _NC_CACHE["nc"] = _build_nc()
    return _NC_CACHE["nc"]


def _prep_inputs(centers1, features, targets, conf_mask):
    f32 = np.float32
    features = np.ascontiguousarray(features, dtype=f32)
    centers1 = np.ascontiguousarray(centers1, dtype=f32).reshape(-1, D)
    targets = np.ascontiguousarray(targets, dtype=f32)
    conf_mask = np.ascontiguousarray(conf_mask, dtype=f32)

    feats_all = np.concatenate([features, centers1], axis=0)  # [N, D]
    fa_pad = np.zeros((NPAD2, D), dtype=f32)
    fa_pad[:N] = feats_all
    fTg_np = np.ascontiguousarray(fa_pad.T).astype(BF)  # [D, NPAD2]

    TA_pad = np.zeros((NPAD2, CP), dtype=f32)
    TA_pad[:B2, :C] = targets
    TA_pad[B2 : B2 + C, :C] = np.eye(C, dtype=f32)
    TAt = TA_pad.reshape(TILES, 128, CP).transpose(1, 0, 2)  # [128, TILES, CP]
    if SWI:
        # per pair: A[CP-1] B[CP-1] ... A[0] B[0] (column-reversed interleave)
        TAg_np = np.empty((128, TILES * CP), f32)
        pairs = TAt.reshape(128, PAIRS, 2, CP)
        TAg_np = TAg_np.reshape(128, PAIRS, 2 * CP)
        TAg_np[:, :, 0::2] = pairs[:, :, 0, ::-1]
        TAg_np[:, :, 1::2] = pairs[:, :, 1, ::-1]
        TAg_np = np.ascontiguousarray(TAg_np.reshape(128, TILES * CP)).astype(F8NP)
    else:
        TAg_np = np.ascontiguousarray(
            TAt.reshape(128, TILES * CP)
        ).astype(F8NP)

    labels = targets.argmax(axis=1)
    cc = targets.sum(axis=0, dtype=np.float64) + 1.0  # [C]
    mpos = np.maximum(cc - 1.0, 1.0)
    W2 = np.where(
        targets.T == 1.0, 1.0 / mpos[:, None], 1.0 / cc[:, None]
    )  # [C, B2] f64
    minv_all = (1.0 / mpos[labels]).astype(f32)  # [B2]

    # -e1[i] = -((ed_i * minv_i + 1) * conf_i - 1) where ed_i replicates the
    # fp8 diagonal exp that entered E on-device: Scalar spline exp + RNE-to-
    # fp8e5 for the half of each 256-row block handled by the Scalar engine,
    # the int8 Schraudolph trick for the Vector half.  The generator per
    # 256-row block is core-uniform: chunk 0 = [ACT, DVE], chunk 1 = [DVE,
    # ACT] (pair parity (p+k)).
    fa_bf = fTg_np.T.astype(np.float32)  # bf16-quantized features [NPAD2, D]
    fsq = (fa_bf[:B2].astype(np.float64) ** 2).sum(axis=1).astype(f32)  # [B2]
    ed_act = (
        np.exp(np.float64(10.0) * fsq.astype(np.float64))
        .astype(f32)
        .astype(F8NP)
        .astype(f32)
    )
    y = np.rint(fsq * np.float32(A_TRICK) + np.float32(B_TRICK))
    ed_dve = (
        np.clip(y, -128, 127).astype(np.int8).view(F8NP).astype(f32)
    )
    # block index within each core's 1024 rows: 0..3 -> generators A,D,D,A
    blk = (np.arange(B2) % R) // 256
    use_act = (blk == 0) | (blk == 3)
    ed = np.where(use_act, ed_act, ed_dve).astype(np.float64)
    e1 = (ed * minv_all.astype(np.float64) + 1.0) * conf_mask.astype(
        np.float64
    ) - 1.0
    ne1_all = (-e1).astype(f32).astype(BF)  # [B2]

    # host linear term: exact f32-feature positive-pair mean logits
    gsum = np.zeros((C, D), dtype=np.float64)
    np.add.at(gsum, labels, features.astype(np.float64))
    gsum += centers1.astype(np.float64)  # class centers are their own class
    feats64 = features.astype(np.float64)
    Sm = (feats64 * gsum[labels]).sum(axis=1) - (feats64 * feats64).sum(axis=1)
    conf64 = conf_mask.astype(np.float64)
    numB = float((conf64 * (1.0 / TEMP) * Sm / mpos[labels]).sum())
    den = float(conf64.sum())

    in_maps = []
    for c in range(CORES):
        rows = slice(c * R, (c + 1) * R)
        fTc_np = np.ascontiguousarray(fTg_np[:, c * R : (c + 1) * R])
        W2c_np = np.ascontiguousarray(
            (W2[:, rows] * conf64[None, rows]).astype(f32)
        ).astype(BF)
        in_maps.append(
            {
                "fTg": fTg_np,
                "TAg": TAg_np,
                "fTc": fTc_np,
                "W2c": W2c_np,
                "ne1": np.ascontiguousarray(ne1_all[rows].reshape(1, R)),
            }
        )
    return in_maps, numB, den


def _run(centers1, features, targets, conf_mask, trace=False, trace_cores=None):
    in_maps, numB, den = _prep_inputs(centers1, features, targets, conf_mask)
    nc = _get_nc()
    kwargs = {}
    if trace:
        # NTFF profiling under axon: shim the (absent) antenv.axon_hooks
        # module and skip the artifact bucket upload.
        import types
        import concourse.bass_utils as bass_utils

        if "antenv.axon_hooks" not in sys.modules:
            mod = types.ModuleType("antenv.axon_hooks")
            mod._hook = None

            def set_axon_ntff_profile_hook(h):
                mod._hook = h

            def get_axon_ntff_profile_hook():
                return mod._hook

            mod.set_axon_ntff_profile_hook = set_axon_ntff_profile_hook
            mod.get_axon_ntff_profile_hook = get_axon_ntff_profile_hook
            sys.modules["antenv.axon_hooks"] = mod
            from trn_agent_boot.trn_boot import _ntff_profile_via_ctypes

            set_axon_ntff_profile_hook(
                _ntff_profile_via_ctypes("/opt/axon/libaxon_pjrt.so")
            )
        bass_utils.upload_artifacts = lambda tmpdir: "local://" + tmpdir
        kwargs = {"trace": True}
        if trace_cores is not None:
            kwargs["trace_cores"] = trace_cores
    res = run_bass_kernel_spmd(nc, in_maps, core_ids=list(range(CORES)), **kwargs)
    numA = 0.0
    for r in res.results:
        s = r["out"][0].astype(np.float64)
        numA += float(np.log(s).sum())
    loss = np.array((numA - numB) / den, dtype=np.float32)
    return loss, res


def kernel(centers1, features, targets, cls_num_list, conf_mask):
    loss, _ = _run(centers1, features, targets, conf_mask)
    return loss


# revision 72
# speedup vs baseline: 1.0022x; 1.0022x over previous
"""Trainium2 Bass kernel for the BalSCL/SSL balanced supervised-contrastive loss.

Distribution: data-parallel over the 8192 anchor rows, 1024 rows per core on
8 NeuronCores.  Each core returns two partial-loss scalars (the conf-weighted
sum of ln S_i over its two 512-row chunks); the host combines them with the
host-computed linear (mean-positive-logit) term and conf denominator.

Math (restructured from the reference, analytically identical):
  N = 8292 columns (8192 anchors + 100 class centers), all unit-norm.
  The row-max subtraction in the reference cancels analytically, so
    loss_i = ln(S_i) - (10/m_i) * Sm_i
  with
    S_i  = sum_{j != i} exp(10 * f_i . g_j) / (cc_j - [lab_j == lab_i])
    Sm_i = sum_{j != i, lab_j == lab_i} f_i . g_j      (host, exact f64)
    m_i  = cc[lab_i] - 1
  Device work per core: raw logits r = fTg.T @ fTc (bf16 PE), elementwise
  exp(10 r) quantized to fp8e5m2, and per-class sums E[c,i] via fp8 DoubleRow
  matmuls (two 128-row j-tiles per PE pass).  S_i = sum_c W2c[c,i] E[c,i] - dg_i
  where W2c folds the per-class balanced weights and the conf mask, and dg
  subtracts the diagonal (j == i) fp8 term bit-exactly.

  The exp work is split between the Scalar engine (true spline exp, RNE to
  fp8e5m2 -- hardware-validated exact) and the Vector engine (Schraudolph
  trick: y = round(r*40/ln2 + B) as int8, bit-reinterpreted as fp8e5m2; B
  calibrated so the mean log error over the logit distribution vanishes).
  j-tile pair p of chunk k goes to Scalar iff (p+k) is even, which maps each
  core's diagonal 256-row blocks to a fixed generator ([ACT, DVE, DVE, ACT]
  over the core's four blocks), so the diagonal correction -e1 can be fully
  precomputed on the host (replicating both generators bit-exactly in numpy)
  and shipped as a bf16 row that the srow matmul folds in via an extra
  contraction row; Ln then reads the PSUM result directly and accumulates
  the conf-masked partial loss.

  The E matmuls use DoubleRowSwInterleave (weights pre-interleaved column-
  reversed on the host) rather than plain DoubleRow: the contiguous weight
  read keeps the 256-column LDWEIGHTS fully hidden under the raw-matmul
  stream (plain DoubleRow exposed ~190 ns per E matmul).
"""

import os
import sys

sys.path.insert(0, "/opt/trn_rl_repo")

import numpy as np
import ml_dtypes

import concourse.bass as bass  # noqa: F401
import concourse.bacc as bacc
import concourse.tile as tile
from concourse import mybir
from concourse.bass_utils import run_bass_kernel_spmd

F32 = mybir.dt.float32
BF16 = mybir.dt.bfloat16
FP8 = mybir.dt.float8e5
I8 = mybir.dt.int8
BF = ml_dtypes.bfloat16
F8NP = ml_dtypes.float8_e5m2
AF = mybir.ActivationFunctionType
ALU = mybir.AluOpType
DR = mybir.MatmulPerfMode.DoubleRow

B2, C, D = 8192, 100, 128
TEMP = 0.1
N = B2 + C
TILES = 66                 # 65 real j-tiles + 1 zero pad (for pairing)
PAIRS = TILES // 2         # 33
NPAD2 = TILES * 128        # 8448
SWI = os.environ.get("KB_SWI", "1") == "1"
CP = 128 if SWI else 112   # padded class count per weight tile
CORES = 8
R = B2 // CORES            # 1024 rows per core
CH = 512                   # i-chunk width (one fp32 PSUM bank)
A_TRICK = 40.0 / np.log(2.0)   # 57.70780163555855
B_TRICK = 59.8                 # calibrated: zero mean log-error (see sim)

_NC_CACHE = {}

EP_BUFS = int(os.environ.get("KB_EPBUFS", "6"))
ELAG = int(os.environ.get("KB_ELAG", "3"))

if os.environ.get("KB_LDWOPT", "0") == "1":
    import concourse.bass_utils as _bu

    _orig_rc = _bu.run_command

    def _rc(cmd, *a, **k):
        cmd = [
            "--enable-ldw-opt=true" if c == "--enable-ldw-opt=false" else c
            for c in cmd
        ]
        return _orig_rc(cmd, *a, **k)

    _bu.run_command = _rc

# Combined exp+ln activation-table set: a single ACT_TABLE_LOAD.
_orig_gat = bacc.get_activation_tables


def _gat_combined(arch):
    tabs = _orig_gat(arch)
    out = {}
    for name, funcs in tabs.items():
        if name in ("exp_and_others", "exp_and_friends", "natural_log"):
            out[name] = set()  # keep position (set ids are positional)
        else:
            out[name] = funcs
    return out


def _build_nc():
    bacc.get_activation_tables = _gat_combined
    try:
        return _build_nc_inner()
    finally:
        bacc.get_activation_tables = _orig_gat


def _build_nc_inner():
    nc = bacc.Bacc()

    fTg = nc.dram_tensor("fTg", [D, NPAD2], BF16, kind="ExternalInput")
    TAg = nc.dram_tensor("TAg", [128, TILES * CP], FP8, kind="ExternalInput")
    fTc = nc.dram_tensor("fTc", [D, R], BF16, kind="ExternalInput")
    W2c = nc.dram_tensor("W2c", [C, R], BF16, kind="ExternalInput")
    ne1 = nc.dram_tensor("ne1", [1, R], BF16, kind="ExternalInput")
    outd = nc.dram_tensor("out", [1, R], F32, kind="ExternalOutput")

    with tile.TileContext(nc) as tc:
        with (
            tc.tile_pool(name="consts", bufs=1) as cp,
            tc.tile_pool(name="expp", bufs=EP_BUFS) as ep,
            tc.tile_pool(name="rawp", bufs=3, space="PSUM") as rp,
            tc.tile_pool(name="epsp", bufs=1, space="PSUM") as pp,
        ):
            # ---------------- input loads (sync queue: big streams) --------
            s_fTc = cp.tile([D, R], BF16)
            s_fTg = cp.tile([D, NPAD2], BF16)
            s_TAg = cp.tile([128, TILES * CP], FP8)
            nc.sync.dma_start(out=s_fTc[:, 0:CH], in_=fTc[:, 0:CH])
            for a, b in (
                (0, 384), (384, 1152), (1152, 2816),
                (2816, 5376), (5376, NPAD2),
            ):
                nc.sync.dma_start(out=s_fTg[:, a:b], in_=fTg[:, a:b])

            # gpsimd queue: memsets + small/medium loads
            s_scr = cp.tile([128, CH], BF16)
            nc.gpsimd.memset(s_scr, 1.0)
            s_ones_bf = cp.tile([128, 1], BF16)
            nc.gpsimd.memset(s_ones_bf, 1.0)
            PW = 2 * CP
            for pa, pb in ((0, 2), (2, 5), (5, 12)):
                nc.gpsimd.dma_start(
                    out=s_TAg[:, pa * PW : pb * PW], in_=TAg[:, pa * PW : pb * PW]
                )
            nc.gpsimd.dma_start(out=s_fTc[:, CH:R], in_=fTc[:, CH:R])
            for pa, pb in ((12, 20), (20, PAIRS)):
                nc.gpsimd.dma_start(
                    out=s_TAg[:, pa * PW : pb * PW], in_=TAg[:, pa * PW : pb * PW]
                )
            s_W2c = cp.tile([C, R], BF16)
            nc.gpsimd.dma_start(out=s_W2c, in_=W2c[:])

            # ---------------- PE warm-up (HAM un-throttle) -----------------
            for w in range(3):
                warmPS = rp.tile([128, 2 * CH], F32, name=f"warm{w}", tag="raw")
                for h in (0, 1):
                    nc.tensor.matmul(
                        warmPS[:, h * CH : (h + 1) * CH],
                        lhsT=s_scr[:, 0:128], rhs=s_scr, start=True, stop=True,
                    )


            # W2E tiles carry an extra row: row C holds -e1 in bf16 (host
            # computed, DMA'd in), so the srow matmul computes
            # S = sum_c W2c*E - e1 directly and Ln reads the PSUM result.
            w2e_t = [
                cp.tile([C + 1, CH], BF16, name=f"W2E{k}", tag=f"W2E{k}")
                for k in (0, 1)
            ]
            for k in (0, 1):
                nc.gpsimd.dma_start(
                    out=w2e_t[k][C : C + 1, :], in_=ne1[:, k * CH : (k + 1) * CH]
                )

            # ---------------- main pipeline --------------------------------
            exps_t = {}

            def raw_pair(k, p):
                rawPS = rp.tile([128, 2 * CH], F32, name=f"raw{k}_{p}", tag="raw")
                for q in (0, 1):
                    t = 2 * p + q
                    nc.tensor.matmul(
                        rawPS[:, q * CH : (q + 1) * CH],
                        lhsT=s_fTg[:, 128 * t : 128 * (t + 1)],
                        rhs=s_fTc[:, k * CH : (k + 1) * CH],
                        start=True,
                        stop=True,
                    )
                return rawPS

            def exp_pair(k, p, rawPS):
                exps = ep.tile([128, 2 * CH], FP8, name=f"exps{k}_{p}", tag="exps")
                if (p + k) % 2 == 0:
                    nc.scalar.activation(
                        out=exps, in_=rawPS, func=AF.Exp, scale=1.0 / TEMP
                    )
                else:
                    nc.vector.tensor_scalar(
                        out=exps[:].bitcast(I8), in0=rawPS,
                        scalar1=A_TRICK, scalar2=B_TRICK,
                        op0=ALU.mult, op1=ALU.add,
                    )
                exps_t[(k, p)] = exps

            EPS_t = [None, None]

            def e_mm(k, p):
                if EPS_t[k] is None:
                    EPS_t[k] = pp.tile([CP, CH], F32, name=f"EPS{k}", tag="EPS")
                exps = exps_t.pop((k, p))
                w_ap = s_TAg[:, 2 * CP * p : 2 * CP * (p + 1)]
                if not SWI:
                    w_ap = w_ap.rearrange("a (two c) -> a two c", two=2)
                nc.tensor.matmul(
                    EPS_t[k],
                    lhsT=w_ap,
                    rhs=exps[:].rearrange("a (two n) -> a two n", two=2),
                    start=(p == 0),
                    stop=(p == PAIRS - 1),
                    perf_mode=(
                        mybir.MatmulPerfMode.DoubleRowSwInterleave if SWI else DR
                    ),
                )

            def mk_w2e(k):
                nc.vector.tensor_mul(
                    w2e_t[k][0:C, :], EPS_t[k][0:C, :],
                    s_W2c[:, k * CH : (k + 1) * CH],
                )

            srow_t = [None, None]

            def mk_srow(k):
                srowPS = pp.tile([1, CH], F32, name=f"srowPS{k}", tag="srow")
                nc.tensor.matmul(
                    srowPS, lhsT=s_ones_bf[0 : C + 1, :], rhs=w2e_t[k],
                    start=True, stop=True,
                )
                srow_t[k] = srowPS

            def mk_out(k):
                # ship per-row S to the host; ln happens there in f64
                s_S = cp.tile([1, CH], F32, name=f"sS{k}", tag=f"sS{k}")
                nc.scalar.copy(s_S, srow_t[k])
                nc.sync.dma_start(
                    out=outd[:, k * CH : (k + 1) * CH], in_=s_S
                )

            sca_after = {(1, 9): [lambda: mk_out(0)]}

            for k in (0, 1):
                for p in range(PAIRS):
                    rawPS = raw_pair(k, p)
                    if k == 1 and p == 3:
                        mk_srow(0)
                    exp_pair(k, p, rawPS)
                    for fn in sca_after.pop((k, p), ()):
                        fn()
                    if p >= ELAG:
                        e_mm(k, p - ELAG)
                for p in range(PAIRS - ELAG, PAIRS):
                    e_mm(k, p)
                mk_w2e(k)

            # ---------------- tail ----------------
            mk_srow(1)
            mk_out(1)

    nc.finalize()
    return nc


def _get_nc():
    if "nc" not in _NC_CACHE:
        _NC_CACHE["nc"] = _build_nc()
    return _NC_CACHE["nc"]


def _prep_inputs(centers1, features, targets, conf_mask):
    f32 = np.float32
    features = np.ascontiguousarray(features, dtype=f32)
    centers1 = np.ascontiguousarray(centers1, dtype=f32).reshape(-1, D)
    targets = np.ascontiguousarray(targets, dtype=f32)
    conf_mask = np.ascontiguousarray(conf_mask, dtype=f32)

    feats_all = np.concatenate([features, centers1], axis=0)  # [N, D]
    fa_pad = np.zeros((NPAD2, D), dtype=f32)
    fa_pad[:N] = feats_all
    fTg_np = np.ascontiguousarray(fa_pad.T).astype(BF)  # [D, NPAD2]

    TA_pad = np.zeros((NPAD2, CP), dtype=f32)
    TA_pad[:B2, :C] = targets
    TA_pad[B2 : B2 + C, :C] = np.eye(C, dtype=f32)
    TAt = TA_pad.reshape(TILES, 128, CP).transpose(1, 0, 2)  # [128, TILES, CP]
    if SWI:
        # per pair: A[CP-1] B[CP-1] ... A[0] B[0] (column-reversed interleave)
        TAg_np = np.empty((128, TILES * CP), f32)
        pairs = TAt.reshape(128, PAIRS, 2, CP)
        TAg_np = TAg_np.reshape(128, PAIRS, 2 * CP)
        TAg_np[:, :, 0::2] = pairs[:, :, 0, ::-1]
        TAg_np[:, :, 1::2] = pairs[:, :, 1, ::-1]
        TAg_np = np.ascontiguousarray(TAg_np.reshape(128, TILES * CP)).astype(F8NP)
    else:
        TAg_np = np.ascontiguousarray(
            TAt.reshape(128, TILES * CP)
        ).astype(F8NP)

    labels = targets.argmax(axis=1)
    cc = targets.sum(axis=0, dtype=np.float64) + 1.0  # [C]
    mpos = np.maximum(cc - 1.0, 1.0)
    W2 = np.where(
        targets.T == 1.0, 1.0 / mpos[:, None], 1.0 / cc[:, None]
    )  # [C, B2] f64
    minv_all = (1.0 / mpos[labels]).astype(f32)  # [B2]

    # -e1[i] = -((ed_i * minv_i + 1) * conf_i - 1) where ed_i replicates the
    # fp8 diagonal exp that entered E on-device: Scalar spline exp + RNE-to-
    # fp8e5 for the half of each 256-row block handled by the Scalar engine,
    # the int8 Schraudolph trick for the Vector half.  The generator per
    # 256-row block is core-uniform: chunk 0 = [ACT, DVE], chunk 1 = [DVE,
    # ACT] (pair parity (p+k)).
    fa_bf = fTg_np.T.astype(np.float32)  # bf16-quantized features [NPAD2, D]
    fsq = (fa_bf[:B2].astype(np.float64) ** 2).sum(axis=1).astype(f32)  # [B2]
    ed_act = (
        np.exp(np.float64(10.0) * fsq.astype(np.float64))
        .astype(f32)
        .astype(F8NP)
        .astype(f32)
    )
    y = np.rint(fsq * np.float32(A_TRICK) + np.float32(B_TRICK))
    ed_dve = (
        np.clip(y, -128, 127).astype(np.int8).view(F8NP).astype(f32)
    )
    # block index within each core's 1024 rows: 0..3 -> generators A,D,D,A
    blk = (np.arange(B2) % R) // 256
    use_act = (blk == 0) | (blk == 3)
    ed = np.where(use_act, ed_act, ed_dve).astype(np.float64)
    e1 = (ed * minv_all.astype(np.float64) + 1.0) * conf_mask.astype(
        np.float64
    ) - 1.0
    ne1_all = (-e1).astype(f32).astype(BF)  # [B2]

    # host linear term: exact f32-feature positive-pair mean logits
    gsum = np.zeros((C, D), dtype=np.float64)
    np.add.at(gsum, labels, features.astype(np.float64))
    gsum += centers1.astype(np.float64)  # class centers are their own class
    feats64 = features.astype(np.float64)
    Sm = (feats64 * gsum[labels]).sum(axis=1) - (feats64 * feats64).sum(axis=1)
    conf64 = conf_mask.astype(np.float64)
    numB = float((conf64 * (1.0 / TEMP) * Sm / mpos[labels]).sum())
    den = float(conf64.sum())

    in_maps = []
    for c in range(CORES):
        rows = slice(c * R, (c + 1) * R)
        fTc_np = np.ascontiguousarray(fTg_np[:, c * R : (c + 1) * R])
        W2c_np = np.ascontiguousarray(
            (W2[:, rows] * conf64[None, rows]).astype(f32)
        ).astype(BF)
        in_maps.append(
            {
                "fTg": fTg_np,
                "TAg": TAg_np,
                "fTc": fTc_np,
                "W2c": W2c_np,
                "ne1": np.ascontiguousarray(ne1_all[rows].reshape(1, R)),
            }
        )
    return in_maps, numB, den


def _run(centers1, features, targets, conf_mask, trace=False, trace_cores=None):
    in_maps, numB, den = _prep_inputs(centers1, features, targets, conf_mask)
    nc = _get_nc()
    kwargs = {}
    if trace:
        # NTFF profiling under axon: shim the (absent) antenv.axon_hooks
        # module and skip the artifact bucket upload.
        import types
        import concourse.bass_utils as bass_utils

        if "antenv.axon_hooks" not in sys.modules:
            mod = types.ModuleType("antenv.axon_hooks")
            mod._hook = None

            def set_axon_ntff_profile_hook(h):
                mod._hook = h

            def get_axon_ntff_profile_hook():
                return mod._hook

            mod.set_axon_ntff_profile_hook = set_axon_ntff_profile_hook
            mod.get_axon_ntff_profile_hook = get_axon_ntff_profile_hook
            sys.modules["antenv.axon_hooks"] = mod
            from trn_agent_boot.trn_boot import _ntff_profile_via_ctypes

            set_axon_ntff_profile_hook(
                _ntff_profile_via_ctypes("/opt/axon/libaxon_pjrt.so")
            )
        bass_utils.upload_artifacts = lambda tmpdir: "local://" + tmpdir
        kwargs = {"trace": True}
        if trace_cores is not None:
            kwargs["trace_cores"] = trace_cores
    res = run_bass_kernel_spmd(nc, in_maps, core_ids=list(range(CORES)), **kwargs)
    numA = 0.0
    for r in res.results:
        s = r["out"][0].astype(np.float64)
        numA += float(np.log(s).sum())
    loss = np.array((numA - numB) / den, dtype=np.float32)
    return loss, res


def kernel(centers1, features, targets, cls_num_list, conf_mask):
    loss, _ = _run(centers1, features, targets, conf_mask)
    return loss


# revision 73
# speedup vs baseline: 1.0066x; 1.0044x over previous
"""Trainium2 Bass kernel for the BalSCL/SSL balanced supervised-contrastive loss.

Distribution: data-parallel over the 8192 anchor rows, 1024 rows per core on
8 NeuronCores.  Each core returns two partial-loss scalars (the conf-weighted
sum of ln S_i over its two 512-row chunks); the host combines them with the
host-computed linear (mean-positive-logit) term and conf denominator.

Math (restructured from the reference, analytically identical):
  N = 8292 columns (8192 anchors + 100 class centers), all unit-norm.
  The row-max subtraction in the reference cancels analytically, so
    loss_i = ln(S_i) - (10/m_i) * Sm_i
  with
    S_i  = sum_{j != i} exp(10 * f_i . g_j) / (cc_j - [lab_j == lab_i])
    Sm_i = sum_{j != i, lab_j == lab_i} f_i . g_j      (host, exact f64)
    m_i  = cc[lab_i] - 1
  Device work per core: raw logits r = fTg.T @ fTc (bf16 PE), elementwise
  exp(10 r) quantized to fp8e5m2, and per-class sums E[c,i] via fp8 DoubleRow
  matmuls (two 128-row j-tiles per PE pass).  S_i = sum_c W2c[c,i] E[c,i] - dg_i
  where W2c folds the per-class balanced weights and the conf mask, and dg
  subtracts the diagonal (j == i) fp8 term bit-exactly.

  The exp work is split between the Scalar engine (true spline exp, RNE to
  fp8e5m2 -- hardware-validated exact) and the Vector engine (Schraudolph
  trick: y = round(r*40/ln2 + B) as int8, bit-reinterpreted as fp8e5m2; B
  calibrated so the mean log error over the logit distribution vanishes).
  j-tile pair p of chunk k goes to Scalar iff (p+k) is even, which maps each
  core's diagonal 256-row blocks to a fixed generator ([ACT, DVE, DVE, ACT]
  over the core's four blocks), so the diagonal correction -e1 can be fully
  precomputed on the host (replicating both generators bit-exactly in numpy)
  and shipped as a bf16 row that the srow matmul folds in via an extra
  contraction row; Ln then reads the PSUM result directly and accumulates
  the conf-masked partial loss.

  The E matmuls use DoubleRowSwInterleave (weights pre-interleaved column-
  reversed on the host) rather than plain DoubleRow: the contiguous weight
  read keeps the 256-column LDWEIGHTS fully hidden under the raw-matmul
  stream (plain DoubleRow exposed ~190 ns per E matmul).
"""

import os
import sys

sys.path.insert(0, "/opt/trn_rl_repo")

import numpy as np
import ml_dtypes

import concourse.bass as bass  # noqa: F401
import concourse.bacc as bacc
import concourse.tile as tile
from concourse import mybir
from concourse.bass_utils import run_bass_kernel_spmd

F32 = mybir.dt.float32
BF16 = mybir.dt.bfloat16
FP8 = mybir.dt.float8e5
I8 = mybir.dt.int8
BF = ml_dtypes.bfloat16
F8NP = ml_dtypes.float8_e5m2
AF = mybir.ActivationFunctionType
ALU = mybir.AluOpType
DR = mybir.MatmulPerfMode.DoubleRow

B2, C, D = 8192, 100, 128
TEMP = 0.1
N = B2 + C
TILES = 66                 # 65 real j-tiles + 1 zero pad (for pairing)
PAIRS = TILES // 2         # 33
NPAD2 = TILES * 128        # 8448
SWI = os.environ.get("KB_SWI", "1") == "1"
CP = 128 if SWI else 112   # padded class count per weight tile
CORES = 8
R = B2 // CORES            # 1024 rows per core
CH = 512                   # i-chunk width (one fp32 PSUM bank)
A_TRICK = 40.0 / np.log(2.0)   # 57.70780163555855
B_TRICK = 59.8                 # calibrated: zero mean log-error (see sim)

_NC_CACHE = {}

EP_BUFS = int(os.environ.get("KB_EPBUFS", "6"))
ELAG = int(os.environ.get("KB_ELAG", "3"))

if os.environ.get("KB_LDWOPT", "0") == "1":
    import concourse.bass_utils as _bu

    _orig_rc = _bu.run_command

    def _rc(cmd, *a, **k):
        cmd = [
            "--enable-ldw-opt=true" if c == "--enable-ldw-opt=false" else c
            for c in cmd
        ]
        return _orig_rc(cmd, *a, **k)

    _bu.run_command = _rc

# Combined exp+ln activation-table set: a single ACT_TABLE_LOAD.
_orig_gat = bacc.get_activation_tables


def _gat_combined(arch):
    tabs = _orig_gat(arch)
    out = {}
    for name, funcs in tabs.items():
        if name in ("exp_and_others", "exp_and_friends", "natural_log"):
            out[name] = set()  # keep position (set ids are positional)
        else:
            out[name] = funcs
    return out


def _build_nc():
    bacc.get_activation_tables = _gat_combined
    try:
        return _build_nc_inner()
    finally:
        bacc.get_activation_tables = _orig_gat


def _build_nc_inner():
    nc = bacc.Bacc()

    fTg = nc.dram_tensor("fTg", [D, NPAD2], BF16, kind="ExternalInput")
    TAg = nc.dram_tensor("TAg", [128, TILES * CP], FP8, kind="ExternalInput")
    fTc = nc.dram_tensor("fTc", [D, R], BF16, kind="ExternalInput")
    W2c = nc.dram_tensor("W2c", [C, R], BF16, kind="ExternalInput")
    ne1 = nc.dram_tensor("ne1", [1, R], BF16, kind="ExternalInput")
    outd = nc.dram_tensor("out", [1, R], F32, kind="ExternalOutput")

    with tile.TileContext(nc) as tc:
        with (
            tc.tile_pool(name="consts", bufs=1) as cp,
            tc.tile_pool(name="expp", bufs=EP_BUFS) as ep,
            tc.tile_pool(name="rawp", bufs=3, space="PSUM") as rp,
            tc.tile_pool(name="epsp", bufs=1, space="PSUM") as pp,
        ):
            # ---------------- input loads (sync queue: big streams) --------
            s_fTc = cp.tile([D, R], BF16)
            s_fTg = cp.tile([D, NPAD2], BF16)
            s_TAg = cp.tile([128, TILES * CP], FP8)
            nc.sync.dma_start(out=s_fTc[:, 0:CH], in_=fTc[:, 0:CH])
            for a, b in (
                (0, 384), (384, 1152), (1152, 2816),
                (2816, 5376), (5376, NPAD2),
            ):
                nc.sync.dma_start(out=s_fTg[:, a:b], in_=fTg[:, a:b])

            # gpsimd queue: memsets + small/medium loads
            s_scr = cp.tile([128, CH], BF16)
            nc.gpsimd.memset(s_scr, 1.0)
            s_ones_bf = cp.tile([128, 1], BF16)
            nc.gpsimd.memset(s_ones_bf, 1.0)
            PW = 2 * CP
            for pa, pb in ((0, 2), (2, 5), (5, 12)):
                nc.gpsimd.dma_start(
                    out=s_TAg[:, pa * PW : pb * PW], in_=TAg[:, pa * PW : pb * PW]
                )
            nc.gpsimd.dma_start(out=s_fTc[:, CH:R], in_=fTc[:, CH:R])
            for pa, pb in ((12, 20), (20, PAIRS)):
                nc.gpsimd.dma_start(
                    out=s_TAg[:, pa * PW : pb * PW], in_=TAg[:, pa * PW : pb * PW]
                )
            s_W2c = cp.tile([C, R], BF16)
            nc.gpsimd.dma_start(out=s_W2c, in_=W2c[:])

            # ---------------- PE warm-up (HAM un-throttle) -----------------
            for w in range(3):
                warmPS = rp.tile([128, 2 * CH], F32, name=f"warm{w}", tag="raw")
                for h in (0, 1):
                    nc.tensor.matmul(
                        warmPS[:, h * CH : (h + 1) * CH],
                        lhsT=s_scr[:, 0:128], rhs=s_scr, start=True, stop=True,
                    )


            # W2E tiles carry an extra row: row C holds -e1 in bf16 (host
            # computed, DMA'd in), so the srow matmul computes
            # S = sum_c W2c*E - e1 directly and Ln reads the PSUM result.
            w2e_t = [
                cp.tile([C + 1, CH], BF16, name=f"W2E{k}", tag=f"W2E{k}")
                for k in (0, 1)
            ]
            for k in (0, 1):
                nc.gpsimd.dma_start(
                    out=w2e_t[k][C : C + 1, :], in_=ne1[:, k * CH : (k + 1) * CH]
                )

            # ---------------- main pipeline --------------------------------
            exps_t = {}

            def raw_pair(k, p):
                rawPS = rp.tile([128, 2 * CH], F32, name=f"raw{k}_{p}", tag="raw")
                for q in (0, 1):
                    t = 2 * p + q
                    nc.tensor.matmul(
                        rawPS[:, q * CH : (q + 1) * CH],
                        lhsT=s_fTg[:, 128 * t : 128 * (t + 1)],
                        rhs=s_fTc[:, k * CH : (k + 1) * CH],
                        start=True,
                        stop=True,
                    )
                return rawPS

            def exp_pair(k, p, rawPS):
                exps = ep.tile([128, 2 * CH], FP8, name=f"exps{k}_{p}", tag="exps")
                if (p + k) % 2 == 0:
                    nc.scalar.activation(
                        out=exps, in_=rawPS, func=AF.Exp, scale=1.0 / TEMP
                    )
                else:
                    nc.vector.tensor_scalar(
                        out=exps[:].bitcast(I8), in0=rawPS,
                        scalar1=A_TRICK, scalar2=B_TRICK,
                        op0=ALU.mult, op1=ALU.add,
                    )
                exps_t[(k, p)] = exps

            EPS_t = [None, None]

            def e_mm(k, p):
                if EPS_t[k] is None:
                    EPS_t[k] = pp.tile([CP, CH], F32, name=f"EPS{k}", tag="EPS")
                exps = exps_t.pop((k, p))
                w_ap = s_TAg[:, 2 * CP * p : 2 * CP * (p + 1)]
                if not SWI:
                    w_ap = w_ap.rearrange("a (two c) -> a two c", two=2)
                nc.tensor.matmul(
                    EPS_t[k],
                    lhsT=w_ap,
                    rhs=exps[:].rearrange("a (two n) -> a two n", two=2),
                    start=(p == 0),
                    stop=(p == stop_p[0]),
                    perf_mode=(
                        mybir.MatmulPerfMode.DoubleRowSwInterleave if SWI else DR
                    ),
                )

            def mk_w2e(k):
                nc.vector.tensor_mul(
                    w2e_t[k][0:C, :], EPS_t[k][0:C, :],
                    s_W2c[:, k * CH : (k + 1) * CH],
                )

            srow_t = [None, None]

            def mk_srow(k):
                srowPS = pp.tile([1, CH], F32, name=f"srowPS{k}", tag="srow")
                nc.tensor.matmul(
                    srowPS, lhsT=s_ones_bf[0 : C + 1, :], rhs=w2e_t[k],
                    start=True, stop=True,
                )
                srow_t[k] = srowPS

            def mk_out(k):
                # ship per-row S to the host; ln happens there in f64
                s_S = cp.tile([1, CH], F32, name=f"sS{k}", tag=f"sS{k}")
                nc.scalar.copy(s_S, srow_t[k])
                nc.sync.dma_start(
                    out=outd[:, k * CH : (k + 1) * CH], in_=s_S
                )

            sca_after = {(1, 9): [lambda: mk_out(0)]}
            stop_p = [PAIRS - 1]

            for k in (0, 1):
                # chunk 1 runs pair 31 last: its exp lands on the faster
                # Scalar generator, and pair 32's Vector exp starts earlier
                order = list(range(PAIRS))
                if k == 1:
                    order = order[:31] + [32, 31]
                stop_p[0] = order[-1]
                for i, p in enumerate(order):
                    rawPS = raw_pair(k, p)
                    if k == 1 and i == 3:
                        mk_srow(0)
                    exp_pair(k, p, rawPS)
                    for fn in sca_after.pop((k, i), ()):
                        fn()
                    if i >= ELAG:
                        e_mm(k, order[i - ELAG])
                for i in range(PAIRS - ELAG, PAIRS):
                    e_mm(k, order[i])
                mk_w2e(k)

            # ---------------- tail ----------------
            mk_srow(1)
            mk_out(1)

    nc.finalize()
    return nc


def _get_nc():
    if "nc" not in _NC_CACHE:
        _NC_CACHE["nc"] = _build_nc()
    return _NC_CACHE["nc"]


def _prep_inputs(centers1, features, targets, conf_mask):
    f32 = np.float32
    features = np.ascontiguousarray(features, dtype=f32)
    centers1 = np.ascontiguousarray(centers1, dtype=f32).reshape(-1, D)
    targets = np.ascontiguousarray(targets, dtype=f32)
    conf_mask = np.ascontiguousarray(conf_mask, dtype=f32)

    feats_all = np.concatenate([features, centers1], axis=0)  # [N, D]
    fa_pad = np.zeros((NPAD2, D), dtype=f32)
    fa_pad[:N] = feats_all
    fTg_np = np.ascontiguousarray(fa_pad.T).astype(BF)  # [D, NPAD2]

    TA_pad = np.zeros((NPAD2, CP), dtype=f32)
    TA_pad[:B2, :C] = targets
    TA_pad[B2 : B2 + C, :C] = np.eye(C, dtype=f32)
    TAt = TA_pad.reshape(TILES, 128, CP).transpose(1, 0, 2)  # [128, TILES, CP]
    if SWI:
        # per pair: A[CP-1] B[CP-1] ... A[0] B[0] (column-reversed interleave)
        TAg_np = np.empty((128, TILES * CP), f32)
        pairs = TAt.reshape(128, PAIRS, 2, CP)
        TAg_np = TAg_np.reshape(128, PAIRS, 2 * CP)
        TAg_np[:, :, 0::2] = pairs[:, :, 0, ::-1]
        TAg_np[:, :, 1::2] = pairs[:, :, 1, ::-1]
        TAg_np = np.ascontiguousarray(TAg_np.reshape(128, TILES * CP)).astype(F8NP)
    else:
        TAg_np = np.ascontiguousarray(
            TAt.reshape(128, TILES * CP)
        ).astype(F8NP)

    labels = targets.argmax(axis=1)
    cc = targets.sum(axis=0, dtype=np.float64) + 1.0  # [C]
    mpos = np.maximum(cc - 1.0, 1.0)
    W2 = np.where(
        targets.T == 1.0, 1.0 / mpos[:, None], 1.0 / cc[:, None]
    )  # [C, B2] f64
    minv_all = (1.0 / mpos[labels]).astype(f32)  # [B2]

    # -e1[i] = -((ed_i * minv_i + 1) * conf_i - 1) where ed_i replicates the
    # fp8 diagonal exp that entered E on-device: Scalar spline exp + RNE-to-
    # fp8e5 for the half of each 256-row block handled by the Scalar engine,
    # the int8 Schraudolph trick for the Vector half.  The generator per
    # 256-row block is core-uniform: chunk 0 = [ACT, DVE], chunk 1 = [DVE,
    # ACT] (pair parity (p+k)).
    fa_bf = fTg_np.T.astype(np.float32)  # bf16-quantized features [NPAD2, D]
    fsq = (fa_bf[:B2].astype(np.float64) ** 2).sum(axis=1).astype(f32)  # [B2]
    ed_act = (
        np.exp(np.float64(10.0) * fsq.astype(np.float64))
        .astype(f32)
        .astype(F8NP)
        .astype(f32)
    )
    y = np.rint(fsq * np.float32(A_TRICK) + np.float32(B_TRICK))
    ed_dve = (
        np.clip(y, -128, 127).astype(np.int8).view(F8NP).astype(f32)
    )
    # block index within each core's 1024 rows: 0..3 -> generators A,D,D,A
    blk = (np.arange(B2) % R) // 256
    use_act = (blk == 0) | (blk == 3)
    ed = np.where(use_act, ed_act, ed_dve).astype(np.float64)
    e1 = (ed * minv_all.astype(np.float64) + 1.0) * conf_mask.astype(
        np.float64
    ) - 1.0
    ne1_all = (-e1).astype(f32).astype(BF)  # [B2]

    # host linear term: exact f32-feature positive-pair mean logits
    gsum = np.zeros((C, D), dtype=np.float64)
    np.add.at(gsum, labels, features.astype(np.float64))
    gsum += centers1.astype(np.float64)  # class centers are their own class
    feats64 = features.astype(np.float64)
    Sm = (feats64 * gsum[labels]).sum(axis=1) - (feats64 * feats64).sum(axis=1)
    conf64 = conf_mask.astype(np.float64)
    numB = float((conf64 * (1.0 / TEMP) * Sm / mpos[labels]).sum())
    den = float(conf64.sum())

    in_maps = []
    for c in range(CORES):
        rows = slice(c * R, (c + 1) * R)
        fTc_np = np.ascontiguousarray(fTg_np[:, c * R : (c + 1) * R])
        W2c_np = np.ascontiguousarray(
            (W2[:, rows] * conf64[None, rows]).astype(f32)
        ).astype(BF)
        in_maps.append(
            {
                "fTg": fTg_np,
                "TAg": TAg_np,
                "fTc": fTc_np,
                "W2c": W2c_np,
                "ne1": np.ascontiguousarray(ne1_all[rows].reshape(1, R)),
            }
        )
    return in_maps, numB, den


def _run(centers1, features, targets, conf_mask, trace=False, trace_cores=None):
    in_maps, numB, den = _prep_inputs(centers1, features, targets, conf_mask)
    nc = _get_nc()
    kwargs = {}
    if trace:
        # NTFF profiling under axon: shim the (absent) antenv.axon_hooks
        # module and skip the artifact bucket upload.
        import types
        import concourse.bass_utils as bass_utils

        if "antenv.axon_hooks" not in sys.modules:
            mod = types.ModuleType("antenv.axon_hooks")
            mod._hook = None

            def set_axon_ntff_profile_hook(h):
                mod._hook = h

            def get_axon_ntff_profile_hook():
                return mod._hook

            mod.set_axon_ntff_profile_hook = set_axon_ntff_profile_hook
            mod.get_axon_ntff_profile_hook = get_axon_ntff_profile_hook
            sys.modules["antenv.axon_hooks"] = mod
            from trn_agent_boot.trn_boot import _ntff_profile_via_ctypes

            set_axon_ntff_profile_hook(
                _ntff_profile_via_ctypes("/opt/axon/libaxon_pjrt.so")
            )
        bass_utils.upload_artifacts = lambda tmpdir: "local://" + tmpdir
        kwargs = {"trace": True}
        if trace_cores is not None:
            kwargs["trace_cores"] = trace_cores
    res = run_bass_kernel_spmd(nc, in_maps, core_ids=list(range(CORES)), **kwargs)
    numA = 0.0
    for r in res.results:
        s = r["out"][0].astype(np.float64)
        numA += float(np.log(s).sum())
    loss = np.array((numA - numB) / den, dtype=np.float32)
    return loss, res


def kernel(centers1, features, targets, cls_num_list, conf_mask):
    loss, _ = _run(centers1, features, targets, conf_mask)
    return loss
